# revision 1
# baseline (speedup 1.0000x reference)
"""BiGraphSAGEDecoder Trainium2 kernel.

Sharding: 8 cores = 4 batches x {up-path, down-path}. One SPMD bass program;
the up/down asymmetry is handled purely by data (down cores receive host-
transposed adjacency / adjacency-weight matrices). Per layer, the two cores of
a batch exchange their halves of the concatenated features with a 2-rank
AllGather, then each normalizes redundantly so both hold the full h.

Math per layer (per core, its path):
  prod = adj . (Wadj . mask + unmask)            (elementwise, DVE)
  s    = prod^T @ h                              (PE, lhsT = prod blocks)
  xT   = (inv @ s)^T                             (PE, rhs = invT streamed)
  cat_own = [x @ Wcat | h @ Wbias_half]          (PE; h@Wb via PE-transposed h)
  exchange cat halves -> full cat; h' = lrelu(cat / max(||cat||, 1e-12))
Layer 2 computes only the two drug rows after s. Head: bilinear form on PE.
"""

import os
import sys
import types
import contextlib

sys.path.insert(0, "/opt/trn_rl_repo")

import numpy as np

import concourse.bass as bass
import concourse.tile as tile
from concourse import mybir, bacc
from concourse.mybir import AxisListType
from concourse.masks import make_identity
from concourse.bass_utils import run_bass_kernel_spmd

FP = mybir.dt.float32
FPR = mybir.dt.float32r
AF = mybir.ActivationFunctionType
ALU = mybir.AluOpType

# ---------------------------------------------------------------------------
# Environment patches (required for this container's toolchain)
# ---------------------------------------------------------------------------


def install_ntff_shim():
    """antenv.axon_hooks is absent in this image; provide it so trace=True
    profiling works (used by test.py, harmless otherwise)."""
    try:
        import antenv.axon_hooks  # noqa: F401
        return
    except ImportError:
        pass
    try:
        import antenv
    except ImportError:
        return
    mod = types.ModuleType("antenv.axon_hooks")
    _holder = {"hook": None}
    mod.set_axon_ntff_profile_hook = lambda h: _holder.__setitem__("hook", h)
    mod.get_axon_ntff_profile_hook = lambda: _holder["hook"]
    sys.modules["antenv.axon_hooks"] = mod
    antenv.axon_hooks = mod
    try:
        from trn_agent_boot.trn_boot import _ntff_profile_via_ctypes

        hook = _ntff_profile_via_ctypes("/opt/axon/libaxon_pjrt.so")
        if hook is not None:
            mod.set_axon_ntff_profile_hook(hook)
    except Exception:
        pass


install_ntff_shim()

if os.environ.get("KGSD_LDW_OPT", "1") != "0":
    # experiment: let walrus dedup back-to-back LDWEIGHTS
    import concourse.bass_utils as _bu
    _orig_run_command = _bu.run_command

    def _patched_run_command(argv, **kw):
        argv = ["--enable-ldw-opt=true" if a == "--enable-ldw-opt=false"
                else a for a in argv]
        return _orig_run_command(argv, **kw)

    _bu.run_command = _patched_run_command

# ---------------------------------------------------------------------------
# Problem constants
# ---------------------------------------------------------------------------

N_FULL = 2048
B = 4
P = 128
DOUT = 256     # per-path cat chunk width
BH = 128       # bias half width per core
DEC = 128
DINS = (256, 768, 768)   # per-layer input dims
EPS = 1e-12
LEAK = 0.1

JSB = 256      # mm1 column superblock (j columns per packed strip tile)
KPACK = 2      # k-tiles packed per mm1 strip tile


def _ceil_div(a, b):
    return -(-a // b)


# ---------------------------------------------------------------------------
# Program builder
# ---------------------------------------------------------------------------

class _StopBuild(Exception):
    pass


def build_program(n_cores: int, N: int = N_FULL, stop_phase: int = 99):
    """Build the SPMD bass program. Returns (nc, input_names).

    stop_phase (debug): 1=x-load only, 2=+l0 bias, 3=+l0 mm1, 4=+l0 mm2,
    5=+l0 mm3+AG+assemble, 6=+l1, 7=full.
    """
    NT = N // P                # k/j/i tiles of 128
    NPAIR = NT // KPACK        # packed k strip-pairs
    NJSB = N // JSB            # mm1 column superblocks
    MM2_JP = 512               # mm2 j' superblock width
    NJP = N // MM2_JP

    nc = bacc.Bacc("TRN2", target_bir_lowering=False, debug=False,
                   num_devices=n_cores)

    # --- DRAM I/O ---
    x_d = nc.dram_tensor("x", [N, DINS[0]], FP, kind="ExternalInput")
    adj_d = nc.dram_tensor("adj", [N, N], FP, kind="ExternalInput")
    invT_d = nc.dram_tensor("invT", [N, N], FP, kind="ExternalInput")
    wa_d = [nc.dram_tensor(f"w{l}a", [N, N], FP, kind="ExternalInput")
            for l in range(3)]
    wc_d = [nc.dram_tensor(f"w{l}c", [DINS[l], DOUT], FP, kind="ExternalInput")
            for l in range(3)]
    wb_d = [nc.dram_tensor(f"w{l}b", [DINS[l], BH], FP, kind="ExternalInput")
            for l in range(3)]
    p1_d = nc.dram_tensor("p1", [3 * DOUT, DEC], FP, kind="ExternalInput")
    p2_d = nc.dram_tensor("p2", [DEC, DEC], FP, kind="ExternalInput")
    y_d = nc.dram_tensor("ypred", [1, 1], FP, kind="ExternalOutput")

    groups = [[i, i + 1] for i in range(0, n_cores, 2)]

    try:
      with tile.TileContext(nc) as tc:
        with contextlib.ExitStack() as ctx:
            # --- pools (all opened once; tags bound memory) ---
            const_p = ctx.enter_context(tc.tile_pool(name="const", bufs=1))
            h_p = ctx.enter_context(tc.tile_pool(name="h", bufs=1))
            # s and hT share one 48KB/partition slot (disjoint lifetimes:
            # hT(l) dies before s(l) is written; s(l) dies before hT(l+1))
            s_p = ctx.enter_context(tc.tile_pool(name="s", bufs=1))
            adj_p = ctx.enter_context(tc.tile_pool(name="adjs", bufs=3))
            w_p = ctx.enter_context(tc.tile_pool(name="ws", bufs=3))
            prod_p = ctx.enter_context(tc.tile_pool(name="prod", bufs=10))
            inv_p = ctx.enter_context(tc.tile_pool(name="invs", bufs=3))
            mm3l_p = ctx.enter_context(tc.tile_pool(name="mm3l", bufs=4))
            wcb_p = ctx.enter_context(tc.tile_pool(name="wcb", bufs=2))
            misc_p = ctx.enter_context(tc.tile_pool(name="misc", bufs=3))
            norm_p = ctx.enter_context(tc.tile_pool(name="norm", bufs=3))
            psum_p = ctx.enter_context(
                tc.tile_pool(name="psum", bufs=8, space="PSUM"))
            dram_p = ctx.enter_context(
                tc.tile_pool(name="dram", bufs=2, space="DRAM"))

            ident = const_p.tile([P, P], FP, tag="ident")
            make_identity(nc, ident)

            # h as per-row-block tiles (pipelines assembly/normalize/use)
            def new_h(din):
                return [h_p.tile([P, din], FPR, tag=f"h{kt}", name="h_t")
                        for kt in range(NT)]

            h_t = new_h(DINS[0])
            # load x -> h tiles
            for kt in range(NT):
                nc.sync.dma_start(
                    h_t[kt][:],
                    x_d.ap()[kt * P:(kt + 1) * P, :].bitcast(FPR))

            drug_rows = None  # final [2,768] tile

            def _dump_and_done(src_ap):
                y_sb0 = misc_p.tile([1, 1], FP, tag="y_sb", name="y_dbg")
                nc.vector.tensor_copy(y_sb0[:], src_ap)
                nc.sync.dma_start(y_d.ap(), y_sb0[:])

            if stop_phase <= 1:
                _dump_and_done(h_t[0][0:1, 0:1])
            n_layers = 0 if stop_phase <= 1 else (
                1 if stop_phase <= 5 else (2 if stop_phase <= 6 else 3))
            for l in range(n_layers):
                din = DINS[l]
                ND = din // P
                last = (l == 2)

                # ---- weights for this layer ----
                wc_t = wcb_p.tile([P, ND * DOUT], FPR, tag="wc")
                for d in range(ND):
                    nc.scalar.dma_start(
                        wc_t[:, d * DOUT:(d + 1) * DOUT],
                        wc_d[l].ap()[d * P:(d + 1) * P, :].bitcast(FPR))
                wb_t = wcb_p.tile([P, ND * BH], FP, tag="wb")
                for d in range(ND):
                    nc.scalar.dma_start(
                        wb_t[:, d * BH:(d + 1) * BH],
                        wb_d[l].ap()[d * P:(d + 1) * P, :])

                # ---- bias chunk: hT = h^T (PE), bias = h @ Wb_half ----
                if not last:
                    # stage DRAM for own cat chunk, split in row halves so
                    # each half's AllGather can overlap the other's compute
                    stage_h = [dram_p.tile([N // 2, DOUT + BH], FP,
                                           tag=f"stage{hh}", name="stage_h")
                               for hh in range(2)]
                    hT_t = s_p.tile([P, ND * N], FP, tag="s")
                    for d in range(ND):
                        for it in range(NT):
                            pt = psum_p.tile([P, P], FP, tag="ps")
                            nc.tensor.transpose(
                                pt[:],
                                h_t[it][:, d * P:(d + 1) * P].bitcast(FP),
                                ident[:])
                            dst = hT_t[:, d * N + it * P: d * N + (it + 1) * P]
                            if it % 2 == 0:
                                nc.vector.tensor_copy(dst, pt[:])
                            else:
                                nc.scalar.copy(dst, pt[:])
                    for it in range(NT):
                        pb = psum_p.tile([P, BH], FP, tag="ps")
                        for d in range(ND):
                            nc.tensor.matmul(
                                pb[:],
                                hT_t[:, d * N + it * P: d * N + (it + 1) * P],
                                wb_t[:, d * BH:(d + 1) * BH],
                                start=(d == 0), stop=(d == ND - 1))
                        sb = misc_p.tile([P, BH], FP, tag="stg_b")
                        nc.scalar.copy(sb[:], pb[:])
                        hh, io = divmod(it, NT // 2)
                        nc.scalar.dma_start(
                            stage_h[hh][io * P:(io + 1) * P, DOUT:DOUT + BH],
                            sb[:])
                else:
                    # only rows N-2, N-1 (partition-0 copy built by l1)
                    hTd = misc_p.tile([P, ND * 2], FP, tag="hTd")
                    for d in range(ND):
                        pt = psum_p.tile([P, 2], FP, tag="ps")
                        nc.tensor.transpose(
                            pt[:],
                            hdrug_t[:, d * P:(d + 1) * P],
                            ident[0:2, 0:2])
                        nc.vector.tensor_copy(hTd[:, d * 2:(d + 1) * 2], pt[:])
                    pb = psum_p.tile([2, BH], FP, tag="ps")
                    for d in range(ND):
                        nc.tensor.matmul(
                            pb[:], hTd[:, d * 2:(d + 1) * 2],
                            wb_t[:, d * BH:(d + 1) * BH],
                            start=(d == 0), stop=(d == ND - 1))
                    stage2 = dram_p.tile([2, DOUT + BH], FP, tag="stage2")
                    sb = misc_p.tile([2, BH], FP, tag="stg_b")
                    nc.scalar.copy(sb[:], pb[:])
                    nc.scalar.dma_start(stage2[:, DOUT:DOUT + BH], sb[:])

                if stop_phase <= 2 and l == 0:
                    _dump_and_done(h_t[0][0:1, 0:1])
                    break

                # ---- mm1: s = prod^T @ h ----
                s_t = s_p.tile([P, NT * din], FPR, tag="s")
                for jsb in range(NJSB):
                    prods = []
                    for t in range(NPAIR):
                        a_t = adj_p.tile([P, KPACK, JSB], FP, tag="adj")
                        nc.sync.dma_start(
                            a_t[:],
                            adj_d.ap()[t * KPACK * P:(t + 1) * KPACK * P,
                                       jsb * JSB:(jsb + 1) * JSB]
                            .rearrange("(a p) c -> p a c", p=P))
                        ww_t = w_p.tile([P, KPACK, JSB], FP, tag="wadj")
                        nc.scalar.dma_start(
                            ww_t[:],
                            wa_d[l].ap()[t * KPACK * P:(t + 1) * KPACK * P,
                                         jsb * JSB:(jsb + 1) * JSB]
                            .rearrange("(a p) c -> p a c", p=P))
                        # mask already baked into Wadj host-side
                        pr_t = prod_p.tile([P, KPACK, JSB], FPR, tag="prod")
                        nc.vector.tensor_tensor(pr_t[:], a_t[:], ww_t[:],
                                                ALU.mult)
                        prods.append(pr_t)
                    for jl in range(JSB // P):
                        j = jsb * (JSB // P) + jl
                        pA = psum_p.tile([P, min(din, 512)], FP, tag="ps")
                        pBw = din - 512
                        pB = psum_p.tile([P, pBw], FP, tag="ps", name="pB") \
                            if pBw > 0 else None
                        for t in range(NPAIR):
                            for a in range(KPACK):
                                k = t * KPACK + a
                                lhsT = prods[t][:, a, jl * P:(jl + 1) * P]
                                st = (k == 0)
                                sp = (k == NT - 1)
                                nc.tensor.matmul(
                                    pA[:], lhsT,
                                    h_t[k][:, 0:min(din, 512)],
                                    start=st, stop=sp)
                                if pB is not None:
                                    nc.tensor.matmul(
                                        pB[:], lhsT,
                                        h_t[k][:, 512:din],
                                        start=st, stop=sp)
                        eng = nc.scalar if (j % 2 == 0) else nc.vector
                        if eng is nc.scalar:
                            nc.scalar.copy(
                                s_t[:, j * din: j * din + min(din, 512)], pA[:])
                            if pB is not None:
                                nc.scalar.copy(
                                    s_t[:, j * din + 512:(j + 1) * din], pB[:])
                        else:
                            nc.vector.tensor_copy(
                                s_t[:, j * din: j * din + min(din, 512)], pA[:])
                            if pB is not None:
                                nc.vector.tensor_copy(
                                    s_t[:, j * din + 512:(j + 1) * din], pB[:])

                if stop_phase <= 3 and l == 0:
                    _dump_and_done(s_t[0:1, 0:1])
                    break

                # ---- mm2: xT = (inv @ s)^T ----
                if not last:
                    xT_dram = dram_p.tile([din, N], FP, tag="xT")
                    for jp in range(NJP):
                        pxs = [psum_p.tile([P, MM2_JP], FP, tag="ps", name="px")
                               for _ in range(ND)]
                        for jt in range(NT):
                            r_t = inv_p.tile([P, MM2_JP], FPR, tag="inv")
                            nc.sync.dma_start(
                                r_t[:],
                                invT_d.ap()[jt * P:(jt + 1) * P,
                                            jp * MM2_JP:(jp + 1) * MM2_JP]
                                .bitcast(FPR))
                            for d in range(ND):
                                nc.tensor.matmul(
                                    pxs[d][:],
                                    s_t[:, jt * din + d * P:
                                        jt * din + (d + 1) * P],
                                    r_t[:],
                                    start=(jt == 0), stop=(jt == NT - 1))
                        for d in range(ND):
                            xs = misc_p.tile([P, MM2_JP], FPR, tag="xstg",
                                             bufs=4)
                            if d % 2 == 0:
                                nc.scalar.copy(xs[:], pxs[d][:])
                            else:
                                nc.vector.tensor_copy(xs[:], pxs[d][:])
                            nc.sync.dma_start(
                                xT_dram[d * P:(d + 1) * P,
                                        jp * MM2_JP:(jp + 1) * MM2_JP]
                                .bitcast(FPR),
                                xs[:])

                    if stop_phase <= 4 and l == 0:
                        _dump_and_done(s_t[0:1, 0:1])
                        break

                    # ---- mm3: cat_own = x @ Wc ----
                    for it in range(NT):
                        pc = psum_p.tile([P, DOUT], FP, tag="ps")
                        for d in range(ND):
                            lt = mm3l_p.tile([P, P], FPR, tag="mm3l")
                            nc.sync.dma_start(
                                lt[:],
                                xT_dram[d * P:(d + 1) * P,
                                        it * P:(it + 1) * P].bitcast(FPR))
                            nc.tensor.matmul(
                                pc[:], lt[:],
                                wc_t[:, d * DOUT:(d + 1) * DOUT],
                                start=(d == 0), stop=(d == ND - 1))
                        sc = misc_p.tile([P, DOUT], FP, tag="stg_c")
                        nc.scalar.copy(sc[:], pc[:])
                        hh, io = divmod(it, NT // 2)
                        nc.scalar.dma_start(
                            stage_h[hh][io * P:(io + 1) * P, 0:DOUT], sc[:])

                    if stop_phase <= 4.3 and l == 0:
                        _dump_and_done(s_t[0:1, 0:1])
                        break

                    # ---- exchange (two halves, overlapped) ----
                    ag_h = []
                    for hh in range(2):
                        agt = dram_p.tile([2, N // 2, DOUT + BH], FP,
                                          tag=f"ag{hh}", name="ag_h")
                        nc.gpsimd.collective_compute(
                            "AllGather", ALU.bypass, replica_groups=groups,
                            ins=[stage_h[hh].opt()], outs=[agt.opt()])
                        ag_h.append(agt)

                    if stop_phase <= 4.6 and l == 0:
                        _dump_and_done(s_t[0:1, 0:1])
                        break

                    # ---- assemble + normalize + lrelu -> new h ----
                    dnext = 3 * DOUT
                    h_t = new_h(dnext)
                    for it in range(NT):
                        hh, io = divmod(it, NT // 2)
                        ag = ag_h[hh]
                        sl = slice(io * P, (io + 1) * P)
                        ht = h_t[it]
                        nc.sync.dma_start(
                            ht[:, 0:DOUT], ag[0, sl, 0:DOUT].bitcast(FPR))
                        nc.sync.dma_start(
                            ht[:, DOUT:2 * DOUT],
                            ag[1, sl, 0:DOUT].bitcast(FPR))
                        nc.sync.dma_start(
                            ht[:, 2 * DOUT:2 * DOUT + BH],
                            ag[0, sl, DOUT:DOUT + BH].bitcast(FPR))
                        nc.sync.dma_start(
                            ht[:, 2 * DOUT + BH:3 * DOUT],
                            ag[1, sl, DOUT:DOUT + BH].bitcast(FPR))
                        ct = ht[:]
                        if stop_phase <= 4.7 and l == 0:
                            continue
                        sq = norm_p.tile([P, dnext], FPR, tag="sq")
                        ssq = norm_p.tile([P, 1], FP, tag="ssq")
                        nc.vector.tensor_tensor(sq[:], ct, ct, ALU.mult)
                        nc.vector.tensor_reduce(ssq[:], sq[:],
                                                AxisListType.X, ALU.add)
                        if stop_phase <= 4.75 and l == 0:
                            continue
                        nrm = norm_p.tile([P, 1], FP, tag="nrm")
                        nc.scalar.activation(nrm[:], ssq[:], AF.Sqrt)
                        nc.vector.tensor_scalar_max(nrm[:], nrm[:], EPS)
                        rn = norm_p.tile([P, 1], FP, tag="rn")
                        nc.vector.reciprocal(rn[:], nrm[:])
                        if stop_phase <= 4.8 and l == 0:
                            continue
                        # h = max(x, 0.1x) with x = cat/norm  (leaky relu)
                        nc.vector.tensor_scalar(ct, ct, rn[:], None, ALU.mult)
                        if stop_phase <= 4.85 and l == 0:
                            continue
                        nc.scalar.mul(sq[:], ct, LEAK)
                        nc.vector.tensor_max(ct, ct, sq[:])
                    if l == 1:
                        # extra partition-0-based copy of the two drug rows
                        # (PE ops cannot address partitions 126:128)
                        hdrug_t = misc_p.tile([2, dnext], FP, tag="hdrug", bufs=1)
                        agl = ag_h[1]
                        NH = N // 2
                        nc.sync.dma_start(hdrug_t[:, 0:DOUT],
                                          agl[0, NH - 2:NH, 0:DOUT])
                        nc.sync.dma_start(hdrug_t[:, DOUT:2 * DOUT],
                                          agl[1, NH - 2:NH, 0:DOUT])
                        nc.sync.dma_start(hdrug_t[:, 2 * DOUT:2 * DOUT + BH],
                                          agl[0, NH - 2:NH, DOUT:DOUT + BH])
                        nc.sync.dma_start(hdrug_t[:, 2 * DOUT + BH:3 * DOUT],
                                          agl[1, NH - 2:NH, DOUT:DOUT + BH])
                        dsq = norm_p.tile([2, dnext], FP, tag="sq")
                        dssq = norm_p.tile([2, 1], FP, tag="ssq")
                        nc.vector.tensor_tensor(dsq[:], hdrug_t[:],
                                                hdrug_t[:], ALU.mult)
                        nc.vector.tensor_reduce(dssq[:], dsq[:],
                                                AxisListType.X, ALU.add)
                        dnrm = norm_p.tile([2, 1], FP, tag="nrm")
                        nc.scalar.activation(dnrm[:], dssq[:], AF.Sqrt)
                        nc.vector.tensor_scalar_max(dnrm[:], dnrm[:], EPS)
                        drn = norm_p.tile([2, 1], FP, tag="rn")
                        nc.vector.reciprocal(drn[:], dnrm[:])
                        nc.vector.tensor_scalar(hdrug_t[:], hdrug_t[:],
                                                drn[:], None, ALU.mult)
                        nc.scalar.mul(dsq[:], hdrug_t[:], LEAK)
                        nc.vector.tensor_max(hdrug_t[:], hdrug_t[:], dsq[:])
                else:
                    # ---- l2: only drug rows j' in {N-2, N-1} ----
                    xT2 = misc_p.tile([P, ND * 2], FP, tag="xT2")
                    px2 = [psum_p.tile([P, 2], FP, tag="ps", name="px2")
                           for _ in range(ND)]
                    for jt in range(NT):
                        r_t = inv_p.tile([P, 2], FPR, tag="inv2")
                        nc.sync.dma_start(
                            r_t[:],
                            invT_d.ap()[jt * P:(jt + 1) * P, N - 2:N]
                            .bitcast(FPR))
                        for d in range(ND):
                            nc.tensor.matmul(
                                px2[d][:],
                                s_t[:, jt * din + d * P:jt * din + (d + 1) * P],
                                r_t[:], start=(jt == 0), stop=(jt == NT - 1))
                    for d in range(ND):
                        nc.vector.tensor_copy(xT2[:, d * 2:(d + 1) * 2],
                                              px2[d][:])
                    pc = psum_p.tile([2, DOUT], FP, tag="ps")
                    for d in range(ND):
                        nc.tensor.matmul(
                            pc[:], xT2[:, d * 2:(d + 1) * 2],
                            wc_t[:, d * DOUT:(d + 1) * DOUT].bitcast(FP),
                            start=(d == 0), stop=(d == ND - 1))
                    sc = misc_p.tile([2, DOUT], FP, tag="stg_c")
                    nc.scalar.copy(sc[:], pc[:])
                    nc.scalar.dma_start(stage2[:, 0:DOUT], sc[:])

                    ag2 = dram_p.tile([2, 2, DOUT + BH], FP, tag="ag2")
                    nc.gpsimd.collective_compute(
                        "AllGather", ALU.bypass, replica_groups=groups,
                        ins=[stage2.opt()], outs=[ag2.opt()])

                    dnext = 3 * DOUT
                    dr = norm_p.tile([2, dnext], FP, tag="drug", bufs=1)
                    nc.sync.dma_start(dr[:, 0:DOUT], ag2[0, :, 0:DOUT])
                    nc.sync.dma_start(dr[:, DOUT:2 * DOUT],
                                      ag2[1, :, 0:DOUT])
                    nc.sync.dma_start(dr[:, 2 * DOUT:2 * DOUT + BH],
                                      ag2[0, :, DOUT:DOUT + BH])
                    nc.sync.dma_start(dr[:, 2 * DOUT + BH:3 * DOUT],
                                      ag2[1, :, DOUT:DOUT + BH])
                    sq = norm_p.tile([2, dnext], FP, tag="sq")
                    ssq = norm_p.tile([2, 1], FP, tag="ssq")
                    nc.vector.tensor_tensor(sq[:], dr[:], dr[:], ALU.mult)
                    nc.vector.tensor_reduce(ssq[:], sq[:],
                                            AxisListType.X, ALU.add)
                    nrm = norm_p.tile([2, 1], FP, tag="nrm")
                    nc.scalar.activation(nrm[:], ssq[:], AF.Sqrt)
                    nc.vector.tensor_scalar_max(nrm[:], nrm[:], EPS)
                    rn = norm_p.tile([2, 1], FP, tag="rn")
                    nc.vector.reciprocal(rn[:], nrm[:])
                    nc.vector.tensor_scalar(dr[:], dr[:], rn[:], None,
                                            ALU.mult)
                    nc.scalar.mul(sq[:], dr[:], LEAK)
                    nc.vector.tensor_max(dr[:], dr[:], sq[:])
                    drug_rows = dr

            if stop_phase <= 6:
                if stop_phase >= 5 and n_layers >= 1:
                    _dump_and_done(h_t[0][0:1, 0:1])
            do_head = stop_phase > 6
            # ---- head: ypred = (a P1 P2) . (b P1) ----
            D3 = 3 * DOUT
            ND3 = D3 // P
            if do_head:
                p1_t = const_p.tile([P, ND3 * DEC], FP, tag="p1")
                for d in range(ND3):
                    nc.sync.dma_start(p1_t[:, d * DEC:(d + 1) * DEC],
                                      p1_d.ap()[d * P:(d + 1) * P, :])
                p2_t = const_p.tile([P, DEC], FP, tag="p2")
                nc.sync.dma_start(p2_t[:], p2_d.ap())
            if do_head:
                dT = misc_p.tile([P, ND3 * 2], FP, tag="dT")
                for d in range(ND3):
                    pt = psum_p.tile([P, 2], FP, tag="ps")
                    nc.tensor.transpose(pt[:], drug_rows[:, d * P:(d + 1) * P],
                                        ident[0:2, 0:2])
                    nc.vector.tensor_copy(dT[:, d * 2:(d + 1) * 2], pt[:])
                pw = psum_p.tile([P, 2], FP, tag="ps")
                for d in range(ND3):
                    nc.tensor.matmul(pw[:], p1_t[:, d * DEC:(d + 1) * DEC],
                                     dT[:, d * 2:(d + 1) * 2],
                                     start=(d == 0), stop=(d == ND3 - 1))
                w_sb = misc_p.tile([P, 2], FP, tag="w_sb")
                nc.vector.tensor_copy(w_sb[:], pw[:])
                ptt = psum_p.tile([P, 1], FP, tag="ps")
                nc.tensor.matmul(ptt[:], p2_t[:], w_sb[:, 0:1], start=True,
                                 stop=True)
                t_sb = misc_p.tile([P, 1], FP, tag="t_sb")
                nc.vector.tensor_copy(t_sb[:], ptt[:])
                py = psum_p.tile([1, 1], FP, tag="ps")
                nc.tensor.matmul(py[:], t_sb[:], w_sb[:, 1:2], start=True,
                                 stop=True)
                y_sb = misc_p.tile([1, 1], FP, tag="y_sb")
                nc.vector.tensor_copy(y_sb[:], py[:])
                nc.sync.dma_start(y_d.ap(), y_sb[:])
    except _StopBuild:
        pass

    nc.compile()
    return nc


# ---------------------------------------------------------------------------
# Host-side input prep
# ---------------------------------------------------------------------------

def make_in_maps(inputs: dict, n_cores: int):
    """Per-core input dicts. Core 2b = up path of batch b, 2b+1 = down."""
    f32c = lambda a: np.ascontiguousarray(np.asarray(a, dtype=np.float32))

    def bake_mask(w):
        w = np.array(w, dtype=np.float32)
        w[-2:, :] = 1.0
        w[:, -2:] = 1.0
        return w
    maps = []
    for c in range(n_cores):
        b, down = divmod(c, 2)
        m = {
            "x": f32c(inputs["x"][b]),
            "p1": f32c(inputs["parameter1"]),
            "p2": f32c(inputs["parameter2"]),
        }
        if not down:
            m["adj"] = f32c(inputs["adj"][b])
            m["invT"] = f32c(inputs["up_inv_deg"][b].T)
            for l in range(3):
                m[f"w{l}a"] = bake_mask(inputs[f"l{l}_up_adj_w"])
                m[f"w{l}c"] = f32c(inputs[f"l{l}_up_w"])
                m[f"w{l}b"] = f32c(inputs[f"l{l}_bias"][:, :BH])
        else:
            m["adj"] = f32c(inputs["adj"][b].T)
            m["invT"] = f32c(inputs["down_inv_deg"][b].T)
            for l in range(3):
                m[f"w{l}a"] = bake_mask(inputs[f"l{l}_down_adj_w"].T)
                m[f"w{l}c"] = f32c(inputs[f"l{l}_down_w"])
                m[f"w{l}b"] = f32c(inputs[f"l{l}_bias"][:, BH:])
        maps.append(m)
    return maps


_nc_cache = {}


def _get_program(n_cores, N):
    key = (n_cores, N)
    if key not in _nc_cache:
        _nc_cache[key] = build_program(n_cores, N)
    return _nc_cache[key]


def kernel(**inputs) -> np.ndarray:
    n_cores = 8
    nc = _get_program(n_cores, N_FULL)
    in_maps = make_in_maps(inputs, n_cores)
    res = run_bass_kernel_spmd(nc, in_maps, core_ids=list(range(n_cores)))
    out = np.zeros((B, 1), dtype=np.float32)
    for b in range(B):
        out[b, 0] = res.results[2 * b]["ypred"][0, 0]
    return out



# revision 17
# speedup vs baseline: 2.0126x; 2.0126x over previous
"""BiGraphSAGEDecoder Trainium2 kernel (v2, fp16).

Sharding: 8 cores = 4 batches x {up-path, down-path}. One SPMD bass program;
up/down asymmetry handled by host-side data prep (down cores get transposed
prod/inv matrices). Per layer the pair exchanges cat halves via a 2-rank
AllGather (fp16 payload; row ssq recomputed after the gather).

Key points vs v1:
  - prod = adj*(Wadj*mask+unmask) premultiplied on host, fp16 (half DMA, no DVE mult)
  - invT loaded once, SBUF-resident for all 3 layers
  - xT stays in SBUF (no DRAM roundtrip)
  - mm2: lhsT = s-block stationary reused for 4x512-wide moving matmuls
  - next-layer hT transposes + bias matmuls emitted into the AG latency gap
  - big batched DMAs
"""

import os
import sys
import types
import contextlib

sys.path.insert(0, "/opt/trn_rl_repo")

import numpy as np

import concourse.bass as bass
import concourse.tile as tile
from concourse import mybir, bacc
from concourse.mybir import AxisListType
from concourse.masks import make_identity
from concourse.bass_utils import run_bass_kernel_spmd

FP = mybir.dt.float32
BF = mybir.dt.float16
AF = mybir.ActivationFunctionType
ALU = mybir.AluOpType

BF_NP = mybir.dt.np(BF)

# ---------------------------------------------------------------------------
# Environment patches (required for this container's toolchain)
# ---------------------------------------------------------------------------


def install_ntff_shim():
    """antenv.axon_hooks is absent in this image; provide it so trace=True
    profiling works (used by test.py, harmless otherwise)."""
    try:
        import antenv.axon_hooks  # noqa: F401
        return
    except ImportError:
        pass
    try:
        import antenv
    except ImportError:
        return
    mod = types.ModuleType("antenv.axon_hooks")
    _holder = {"hook": None}
    mod.set_axon_ntff_profile_hook = lambda h: _holder.__setitem__("hook", h)
    mod.get_axon_ntff_profile_hook = lambda: _holder["hook"]
    sys.modules["antenv.axon_hooks"] = mod
    antenv.axon_hooks = mod
    try:
        from trn_agent_boot.trn_boot import _ntff_profile_via_ctypes

        hook = _ntff_profile_via_ctypes("/opt/axon/libaxon_pjrt.so")
        if hook is not None:
            mod.set_axon_ntff_profile_hook(hook)
    except Exception:
        pass


install_ntff_shim()

# NOTE: walrus --enable-ldw-opt is incompatible with the standalone
# InstLdweights that 16-bit matmuls legalize into; keep the default (false).

# ---------------------------------------------------------------------------
# Problem constants
# ---------------------------------------------------------------------------

N_FULL = 2048
B = 4
P = 128
DOUT = 256     # per-path cat chunk width
BH = 128       # bias half width per core
DEC = 128
DINS = (256, 768, 768)   # per-layer input dims
EPS = 1e-12
LEAK = 0.1
STW = DOUT + BH + 2      # stage row width: 256 cat + 128 bias + 2 ssq(f32)


class _StopBuild(Exception):
    pass


def build_program(n_cores: int, N: int = N_FULL, stop: float = None,
                  ssq_mode: str = None):
    if stop is None:
        stop = float(os.environ.get("KGSD_STOP", "99"))
    if ssq_mode is None:
        ssq_mode = os.environ.get("KGSD_SSQ", "renorm")
    stw = DOUT + BH + (2 if ssq_mode == "bitcast" else 0)
    NT = N // P            # 128-row tiles of N
    NH = NT // 2           # it-tiles per half
    JSB = 2 * P            # mm1 j-superblock columns
    NJSB = N // JSB
    NPAIR = 2              # k-tiles packed per prod strip
    NKS = NT // NPAIR      # prod strips per jsb
    MVC = 512              # max moving free dim
    NJC = max(N // MVC, 1) # mm2 column chunks
    CW = min(N, MVC)       # mm2 chunk width

    nc = bacc.Bacc("TRN2", target_bir_lowering=False, debug=False,
                   num_devices=n_cores)

    # --- DRAM I/O (all bf16 except head params / output) ---
    x_d = nc.dram_tensor("x", [N, DINS[0]], BF, kind="ExternalInput")
    xT_d = nc.dram_tensor("xT", [DINS[0], N], BF, kind="ExternalInput")
    prod_d = [nc.dram_tensor(f"prod{l}", [N, N], BF, kind="ExternalInput")
              for l in range(3)]
    invT_d = nc.dram_tensor("invT", [N, N], BF, kind="ExternalInput")
    wc_d = [nc.dram_tensor(f"w{l}c", [DINS[l], DOUT], BF, kind="ExternalInput")
            for l in range(3)]
    wb_d = [nc.dram_tensor(f"w{l}b", [DINS[l], BH], BF, kind="ExternalInput")
            for l in range(3)]
    p1_d = nc.dram_tensor("p1", [3 * DOUT, DEC], FP, kind="ExternalInput")
    p2_d = nc.dram_tensor("p2", [DEC, DEC], FP, kind="ExternalInput")
    y_d = nc.dram_tensor("ypred", [1, 1], FP, kind="ExternalOutput")

    groups = [[i, i + 1] for i in range(0, n_cores, 2)]

    with tile.TileContext(nc) as tc:
      with contextlib.ExitStack() as ctx:
       try:
        const_p = ctx.enter_context(tc.tile_pool(name="const", bufs=1))
        h_p = ctx.enter_context(tc.tile_pool(name="h", bufs=1))
        sht_p = ctx.enter_context(tc.tile_pool(name="sht", bufs=1))
        xt_p = ctx.enter_context(tc.tile_pool(name="xt", bufs=1))
        inv_p = ctx.enter_context(tc.tile_pool(name="inv", bufs=1))
        prod_p = ctx.enter_context(tc.tile_pool(name="prod", bufs=8))
        stage_p = ctx.enter_context(tc.tile_pool(name="stg", bufs=1))
        w_p = ctx.enter_context(tc.tile_pool(name="w", bufs=2))
        misc_p = ctx.enter_context(tc.tile_pool(name="misc", bufs=1))
        norm_p = ctx.enter_context(tc.tile_pool(name="norm", bufs=2))
        psum_p = ctx.enter_context(
            tc.tile_pool(name="psum", bufs=8, space="PSUM"))
        dram_p = ctx.enter_context(
            tc.tile_pool(name="dram", bufs=1, space="DRAM"))

        identB = const_p.tile([P, P], BF, tag="identB")
        make_identity(nc, identB)
        identF = const_p.tile([P, P], FP, tag="identF")
        make_identity(nc, identF)

        # --- resident loads ---
        inv_t = inv_p.tile([P, NT, N], BF, tag="invT")
        nc.sync.dma_start(
            inv_t[:], invT_d.ap().rearrange("(a p) c -> p a c", p=P))

        def new_h(tag, din):
            return h_p.tile([P, NT, din], BF, tag=tag, name="h_t")

        h_t = new_h("h_even", DINS[0])
        nc.sync.dma_start(
            h_t[:], x_d.ap().rearrange("(a p) c -> p a c", p=P))
        # layer-0 hT comes transposed from the host
        hT_t = sht_p.tile([P, 2, N], BF, tag="s_hT", name="hT_t")
        nc.sync.dma_start(
            hT_t[:], xT_d.ap().rearrange("(d p) c -> p d c", p=P))

        p1_t = const_p.tile([P, 3 * DOUT // P, DEC], FP, tag="p1")
        nc.scalar.dma_start(
            p1_t[:], p1_d.ap().rearrange("(d p) c -> p d c", p=P))
        p2_t = const_p.tile([P, DEC], FP, tag="p2")
        nc.scalar.dma_start(p2_t[:], p2_d.ap())

        def load_w(l):
            ND = DINS[l] // P
            wc_t = w_p.tile([P, ND, DOUT], BF, tag="wc", name="wc_t")
            nc.scalar.dma_start(
                wc_t[:, 0:ND, :],
                wc_d[l].ap().rearrange("(d p) c -> p d c", p=P))
            wb_t = w_p.tile([P, ND, BH], BF, tag="wb", name="wb_t")
            nc.scalar.dma_start(
                wb_t[:, 0:ND, :],
                wb_d[l].ap().rearrange("(d p) c -> p d c", p=P))
            return wc_t, wb_t

        w_cur = load_w(0)
        STOP1 = stop  # checkpoint granularity knob

        # rotating evacuation engine
        ev = [0]

        def evac(dst, src):
            ev[0] += 1
            if ev[0] % 2 == 0:
                nc.vector.tensor_copy(dst, src)
            else:
                nc.scalar.copy(dst, src)

        def _dump(src_ap):
            y_dbg = misc_p.tile([1, 1], FP, tag="y_sb", name="y_dbg")
            nc.vector.tensor_copy(y_dbg[:], src_ap)
            nc.sync.dma_start(y_d.ap(), y_dbg[:])
            raise _StopBuild

        # --------------- per-layer helpers ---------------

        def bias_mms(hT, wb_t, ND, its, stage_sb):
            """stage[:, io, DOUT:DOUT+BH] = (h @ Wb_half) rows for given its."""
            for it in its:
                pb = psum_p.tile([P, BH], FP, tag="ps", name="pb")
                for d in range(ND):
                    nc.tensor.matmul(
                        pb[:], hT[:, d, it * P:(it + 1) * P],
                        wb_t[:, d, :], start=(d == 0), stop=(d == ND - 1))
                half, io = divmod(it, NH)
                evac(stage_sb[half][:, io, DOUT:DOUT + BH], pb[:])

        def mm1(l, din, ND, s_t, h_t):
            """s = prod^T @ h."""
            c1 = min(din, MVC)
            for jsb in range(NJSB):
                strips = []
                for t in range(NKS):
                    pr = prod_p.tile([P, NPAIR, JSB], BF, tag="prod",
                                     name="pr")
                    nc.sync.dma_start(
                        pr[:],
                        prod_d[l].ap()[t * NPAIR * P:(t + 1) * NPAIR * P,
                                       jsb * JSB:(jsb + 1) * JSB]
                        .rearrange("(a p) c -> p a c", p=P))
                    strips.append(pr)
                for jl in range(JSB // P):
                    j = jsb * (JSB // P) + jl
                    pA = psum_p.tile([P, c1], FP, tag="ps", name="pA")
                    pB = psum_p.tile([P, din - c1], FP, tag="ps", name="pB") \
                        if din > c1 else None
                    for t in range(NKS):
                        for a in range(NPAIR):
                            k = t * NPAIR + a
                            lhsT = strips[t][:, a, jl * P:(jl + 1) * P]
                            st, sp = (k == 0), (k == NT - 1)
                            nc.tensor.matmul(pA[:], lhsT, h_t[:, k, 0:c1],
                                             start=st, stop=sp)
                            if pB is not None:
                                nc.tensor.matmul(pB[:], lhsT,
                                                 h_t[:, k, c1:din],
                                                 start=st, stop=sp)
                    evac(s_t[:, j, 0:c1], pA[:])
                    if pB is not None:
                        evac(s_t[:, j, c1:din], pB[:])

        def mm2_half(half, din, ND, s_t, xT_t):
            """xT[:, d, half cols] = ((inv @ s)^T) for one column half."""
            jcs = range(half * (NJC // 2), (half + 1) * (NJC // 2)) \
                if NJC > 1 else range(1)
            jcs = list(jcs)
            for d in range(ND):
                pxs = []
                for jc in jcs:
                    pxs.append(psum_p.tile([P, CW], FP, tag="ps", name="px"))
                for jt in range(NT):
                    lhsT = s_t[:, jt, d * P:(d + 1) * P]
                    st, sp = (jt == 0), (jt == NT - 1)
                    for i, jc in enumerate(jcs):
                        nc.tensor.matmul(
                            pxs[i][:], lhsT,
                            inv_t[:, jt, jc * CW:(jc + 1) * CW],
                            start=st, stop=sp)
                for i, jc in enumerate(jcs):
                    evac(xT_t[:, d, jc * CW:(jc + 1) * CW], pxs[i][:])

        def mm3_half(half, ND, xT_t, wc_t, stage_sb):
            for io in range(NH):
                it = half * NH + io
                pc = psum_p.tile([P, DOUT], FP, tag="ps", name="pc")
                for d in range(ND):
                    nc.tensor.matmul(
                        pc[:], xT_t[:, d, it * P:(it + 1) * P],
                        wc_t[:, d, :], start=(d == 0), stop=(d == ND - 1))
                evac(stage_sb[half][:, io, 0:DOUT], pc[:])

        def ssq_half(half, stage_sb):
            if ssq_mode != "bitcast":
                return
            for io in range(NH):
                scr = norm_p.tile([P, DOUT + BH], FP, tag="sqscr",
                                  name="scr")
                nc.vector.tensor_tensor_reduce(
                    out=scr[:],
                    in0=stage_sb[half][:, io, 0:DOUT + BH],
                    in1=stage_sb[half][:, io, 0:DOUT + BH],
                    scale=1.0, scalar=0.0,
                    op0=ALU.mult, op1=ALU.add,
                    accum_out=stage_sb[half][:, io,
                                             DOUT + BH:DOUT + BH + 2]
                    .bitcast(FP))

        def norm_half(half, h_new, din_new, ag, nrm_pool):
            """readback AG -> h_new[:, half], normalize + leaky relu."""
            hs = slice(half * NH, (half + 1) * NH)
            nc.scalar.dma_start(
                h_new[:, hs, 0:DOUT],
                ag[0, :, 0:DOUT].rearrange("(a p) c -> p a c", p=P))
            nc.scalar.dma_start(
                h_new[:, hs, DOUT:2 * DOUT],
                ag[1, :, 0:DOUT].rearrange("(a p) c -> p a c", p=P))
            nc.scalar.dma_start(
                h_new[:, hs, 2 * DOUT:2 * DOUT + BH],
                ag[0, :, DOUT:DOUT + BH]
                .rearrange("(a p) c -> p a c", p=P))
            nc.scalar.dma_start(
                h_new[:, hs, 2 * DOUT + BH:3 * DOUT],
                ag[1, :, DOUT:DOUT + BH]
                .rearrange("(a p) c -> p a c", p=P))
            if stop <= 4.6:
                return
            ssq = nrm_pool.tile([P, NH], FP, tag="ssq", name="ssq")
            if ssq_mode == "bitcast":
                ssa = nrm_pool.tile([P, NH], FP, tag="ssa", name="ssa")
                ssb = nrm_pool.tile([P, NH], FP, tag="ssb", name="ssb")
                nc.scalar.dma_start(
                    ssa[:],
                    ag[0, :, DOUT + BH:DOUT + BH + 2].bitcast(FP)
                    .rearrange("(a p) c -> p (a c)", p=P))
                nc.scalar.dma_start(
                    ssb[:],
                    ag[1, :, DOUT + BH:DOUT + BH + 2].bitcast(FP)
                    .rearrange("(a p) c -> p (a c)", p=P))
                nc.vector.tensor_tensor(ssq[:], ssa[:], ssb[:], ALU.add)
            else:
                # recompute ssq from assembled rows (square into bf16
                # scratch, then reduce; InstTensorTensorReduce faults on
                # this hardware path)
                for io in range(NH):
                    it = half * NH + io
                    scr = nrm_pool.tile([P, din_new], mybir.dt.bfloat16,
                                        tag="sqscr", name="scr")
                    nc.vector.tensor_tensor(scr[:], h_new[:, it, :],
                                            h_new[:, it, :], ALU.mult)
                    nc.vector.tensor_reduce(ssq[:, io:io + 1], scr[:],
                                            AxisListType.X, ALU.add)
            if stop <= 4.7:
                return
            nrm = nrm_pool.tile([P, NH], FP, tag="nrm", name="nrm")
            nc.scalar.activation(nrm[:], ssq[:], AF.Sqrt)
            nc.vector.tensor_scalar_max(nrm[:], nrm[:], EPS)
            rn = nrm_pool.tile([P, NH], FP, tag="rn", name="rn")
            nc.vector.reciprocal(rn[:], nrm[:])
            rn01 = nrm_pool.tile([P, NH], FP, tag="rn01", name="rn01")
            nc.vector.tensor_scalar_mul(rn01[:], rn[:], LEAK)
            if stop <= 4.8:
                return
            for io in range(NH):
                it = half * NH + io
                c = h_new[:, it, :]
                tmp = norm_p.tile([P, 768], BF, tag="tmp", name="tmp")
                tv = tmp[:, 0:din_new]
                nc.scalar.activation(tv, c, AF.Copy,
                                     scale=rn01[:, io:io + 1])
                if stop <= 4.85:
                    continue
                nc.vector.tensor_scalar(c, c, rn[:, io:io + 1], None,
                                        ALU.mult)
                if stop <= 4.9:
                    continue
                nc.vector.tensor_tensor(c, c, tv, ALU.max)

        def transposes_half(half, h_new, NDn, hT_new):
            for io in range(NH):
                it = half * NH + io
                for d in range(NDn):
                    pt = psum_p.tile([P, P], BF, tag="ps", name="pt")
                    nc.tensor.transpose(
                        pt[:], h_new[:, it, d * P:(d + 1) * P], identB[:])
                    evac(hT_new[:, d, it * P:(it + 1) * P], pt[:])

        # =================== layers 0 and 1 ===================

        if stop <= 1:
            _dump(h_t[0:1, 0, 0:1])

        stage_sb = [stage_p.tile([P, NH, stw], BF, tag=f"stage{hh}",
                                 name="stage_sb") for hh in range(2)]

        for l in range(2):
            din = DINS[l]
            ND = din // P
            NDn = DINS[l + 1] // P
            w_next = load_w(l + 1)
            wc_t, wb_t = w_cur

            if l == 0:
                # layer-0 bias for all rows (hT_t = host xT)
                bias_mms(hT_t, wb_t, ND, range(NT), stage_sb)
                if stop <= 2:
                    _dump(stage_sb[0][0:1, 0, DOUT:DOUT + 1])

            s_t = sht_p.tile([P, NT, din], BF, tag="s_hT", name="s_t")
            mm1(l, din, ND, s_t, h_t)
            if stop <= 3 and l == 0:
                _dump(s_t[0:1, 0, 0:1])

            xT_t = xt_p.tile([P, ND, N], BF, tag="xT", name="xT_t")

            stage_d = [dram_p.tile([NH * P, stw], BF, tag=f"stgd{hh}",
                                   name="stage_d") for hh in range(2)]
            ag_d = [dram_p.tile([2, NH * P, stw], BF, tag=f"ag{hh}",
                                name="ag_d") for hh in range(2)]

            h_new = new_h("h_odd" if l == 0 else "h_even", DINS[l + 1])
            hT_new = sht_p.tile([P, NDn, N], BF, tag="s_hT", name="hT_new")

            for half in range(2):
                mm2_half(half, din, ND, s_t, xT_t)
                mm3_half(half, ND, xT_t, wc_t, stage_sb)
                ssq_half(half, stage_sb)
                nc.scalar.dma_start(
                    stage_d[half].rearrange("(a p) c -> p a c", p=P),
                    stage_sb[half][:])
                if stop > 4.2:
                    nc.gpsimd.collective_compute(
                        "AllGather", ALU.bypass, replica_groups=groups,
                        ins=[stage_d[half].opt()], outs=[ag_d[half].opt()])

            if stop <= 4.2 and l == 0:
                _dump(xT_t[0:1, 0, 0:1])
            if stop <= 4.5 and l == 0:
                _dump(xT_t[0:1, 0, 0:1])

            for half in range(2):
                norm_half(half, h_new, DINS[l + 1], ag_d[half], norm_p)
                if l == 0 and stop > 4.95:
                    # fill AG latency with next-layer transposes + bias
                    transposes_half(half, h_new, NDn, hT_new)
                    bias_mms(hT_new, w_next[1], NDn,
                             range(half * NH, (half + 1) * NH), stage_sb)

            if stop <= 5 and l == 0:
                _dump(h_new[0:1, 0, 0:1])

            h_t = h_new
            hT_t = hT_new
            w_cur = w_next

        # =================== layer 2 (drug rows only) ===================

        if stop <= 6:
            _dump(h_t[0:1, 0, 0:1])

        din = DINS[2]
        ND = din // P
        wc_t, wb_t = w_cur

        # hTd = (h rows N-2, N-1)^T: transpose the last h row-tile and keep
        # columns 126:128 (no cross-partition DMA needed)
        hTd = misc_p.tile([P, ND, 2], BF, tag="hTd", name="hTd")
        for d in range(ND):
            pt = psum_p.tile([P, P], BF, tag="ps", name="ptd")
            nc.tensor.transpose(pt[:], h_t[:, NT - 1, d * P:(d + 1) * P],
                                identB[:])
            nc.vector.tensor_copy(hTd[:, d, :], pt[:, P - 2:P])
        pb2 = psum_p.tile([2, BH], FP, tag="ps", name="pb2")
        for d in range(ND):
            nc.tensor.matmul(pb2[:], hTd[:, d, :], wb_t[:, d, :],
                             start=(d == 0), stop=(d == ND - 1))
        stage2_sb = misc_p.tile([2, DOUT + BH], FP, tag="st2dr", name="st2")
        nc.scalar.copy(stage2_sb[:, DOUT:DOUT + BH], pb2[:])

        s_t = sht_p.tile([P, NT, din], BF, tag="s_hT", name="s2_t")
        mm1(2, din, ND, s_t, h_t)
        if stop <= 6.5:
            _dump(s_t[0:1, 0, 0:1])

        # mm2 drug: x_drug[2, din] = (inv @ s)[N-2:N, :]
        c1 = min(din, MVC)
        px2a = psum_p.tile([2, c1], FP, tag="ps", name="px2a")
        px2b = psum_p.tile([2, din - c1], FP, tag="ps", name="px2b") \
            if din > c1 else None
        for jt in range(NT):
            lhsT = inv_t[:, jt, N - 2:N]
            st, sp = (jt == 0), (jt == NT - 1)
            nc.tensor.matmul(px2a[:], lhsT, s_t[:, jt, 0:c1],
                             start=st, stop=sp)
            if px2b is not None:
                nc.tensor.matmul(px2b[:], lhsT, s_t[:, jt, c1:din],
                                 start=st, stop=sp)
        xd = misc_p.tile([2, din], BF, tag="hdrug", name="xd")
        nc.vector.tensor_copy(xd[:, 0:c1], px2a[:])
        if px2b is not None:
            nc.scalar.copy(xd[:, c1:din], px2b[:])
        xT2 = misc_p.tile([P, ND, 2], BF, tag="xT2", name="xT2")
        for d in range(ND):
            pt = psum_p.tile([P, 2], BF, tag="ps", name="ptx")
            nc.tensor.transpose(pt[:], xd[:, d * P:(d + 1) * P], identB[0:2, 0:2])
            nc.vector.tensor_copy(xT2[:, d, :], pt[:])
        pc2 = psum_p.tile([2, DOUT], FP, tag="ps", name="pc2")
        for d in range(ND):
            nc.tensor.matmul(pc2[:], xT2[:, d, :], wc_t[:, d, :],
                             start=(d == 0), stop=(d == ND - 1))
        nc.scalar.copy(stage2_sb[:, 0:DOUT], pc2[:])

        if stop <= 6.8:
            _dump(stage2_sb[0:1, 0:1])

        stage2_d = dram_p.tile([2, DOUT + BH], FP, tag="stg2d",
                               name="stage2_d")
        nc.scalar.dma_start(stage2_d[:], stage2_sb[:])
        ag2_d = dram_p.tile([2, 2, DOUT + BH], FP, tag="ag2d", name="ag2_d")
        nc.gpsimd.collective_compute(
            "AllGather", ALU.bypass, replica_groups=groups,
            ins=[stage2_d.opt()], outs=[ag2_d.opt()])

        D3 = 3 * DOUT
        dr = misc_p.tile([2, D3], FP, tag="st2dr", name="dr")
        nc.scalar.dma_start(dr[:, 0:DOUT], ag2_d[0, :, 0:DOUT])
        nc.scalar.dma_start(dr[:, DOUT:2 * DOUT], ag2_d[1, :, 0:DOUT])
        nc.scalar.dma_start(dr[:, 2 * DOUT:2 * DOUT + BH],
                            ag2_d[0, :, DOUT:DOUT + BH])
        nc.scalar.dma_start(dr[:, 2 * DOUT + BH:D3],
                            ag2_d[1, :, DOUT:DOUT + BH])
        dssq = misc_p.tile([2, 1], FP, tag="dssq", name="dssq")
        dscr = norm_p.tile([2, D3], mybir.dt.bfloat16, tag="sqscr",
                           name="dscr")
        nc.vector.tensor_tensor(dscr[:], dr[:], dr[:], ALU.mult)
        nc.vector.tensor_reduce(dssq[:], dscr[:], AxisListType.X, ALU.add)
        dnrm = misc_p.tile([2, 1], FP, tag="dnrm", name="dnrm")
        nc.scalar.activation(dnrm[:], dssq[:], AF.Sqrt)
        nc.vector.tensor_scalar_max(dnrm[:], dnrm[:], EPS)
        drn = misc_p.tile([2, 1], FP, tag="drn", name="drn")
        nc.vector.reciprocal(drn[:], dnrm[:])
        drn01 = misc_p.tile([2, 1], FP, tag="drn01", name="drn01")
        nc.vector.tensor_scalar_mul(drn01[:], drn[:], LEAK)
        dtmp = norm_p.tile([2, D3], BF, tag="tmp", name="dtmp")
        nc.scalar.activation(dtmp[:], dr[:], AF.Copy, scale=drn01[:])
        nc.vector.tensor_scalar(dr[:], dr[:], drn[:], None, ALU.mult)
        nc.vector.tensor_max(dr[:], dr[:], dtmp[:])

        # ---- head: ypred = (a P1 P2) . (b P1) ----
        ND3 = D3 // P
        dT = misc_p.tile([P, ND3, 2], FP, tag="dT", name="dT")
        for d in range(ND3):
            pt = psum_p.tile([P, 2], FP, tag="ps", name="pth")
            nc.tensor.transpose(pt[:], dr[:, d * P:(d + 1) * P], identF[0:2, 0:2])
            nc.vector.tensor_copy(dT[:, d, :], pt[:])
        pw = psum_p.tile([P, 2], FP, tag="ps", name="pw")
        for d in range(ND3):
            nc.tensor.matmul(pw[:], p1_t[:, d, :], dT[:, d, :],
                             start=(d == 0), stop=(d == ND3 - 1))
        w_sb = misc_p.tile([P, 2], FP, tag="w_sb", name="w_sb")
        nc.vector.tensor_copy(w_sb[:], pw[:])
        ptt = psum_p.tile([P, 1], FP, tag="ps", name="ptt")
        nc.tensor.matmul(ptt[:], p2_t[:], w_sb[:, 0:1], start=True, stop=True)
        t_sb = misc_p.tile([P, 1], FP, tag="t_sb", name="t_sb")
        nc.vector.tensor_copy(t_sb[:], ptt[:])
        py = psum_p.tile([1, 1], FP, tag="ps", name="py")
        nc.tensor.matmul(py[:], t_sb[:], w_sb[:, 1:2], start=True, stop=True)
        y_sb = misc_p.tile([1, 1], FP, tag="y_sb", name="y_sb")
        nc.vector.tensor_copy(y_sb[:], py[:])
        nc.sync.dma_start(y_d.ap(), y_sb[:])
       except _StopBuild:
        pass

    nc.compile()
    return nc


# ---------------------------------------------------------------------------
# Host-side input prep
# ---------------------------------------------------------------------------

def make_in_maps(inputs: dict, n_cores: int):
    """Per-core input dicts. Core 2b = up path of batch b, 2b+1 = down."""
    N = np.asarray(inputs["adj"]).shape[1]

    def prodw(w):
        # W*mask + unmask with mask zero on last-2 rows/cols
        w = np.array(w, dtype=np.float32)
        w[-2:, :] = 1.0
        w[:, -2:] = 1.0
        return w

    wup = [prodw(inputs[f"l{l}_up_adj_w"]) for l in range(3)]
    wdn = [prodw(inputs[f"l{l}_down_adj_w"]) for l in range(3)]

    maps = []
    for c in range(n_cores):
        b, down = divmod(c, 2)
        adj = np.asarray(inputs["adj"][b], dtype=np.float32)
        x = np.asarray(inputs["x"][b], dtype=np.float32)
        m = {
            "x": x.astype(BF_NP),
            "xT": np.ascontiguousarray(x.T).astype(BF_NP),
            "p1": np.asarray(inputs["parameter1"], dtype=np.float32),
            "p2": np.asarray(inputs["parameter2"], dtype=np.float32),
        }
        if not down:
            for l in range(3):
                m[f"prod{l}"] = (adj * wup[l]).astype(BF_NP)
            m["invT"] = np.ascontiguousarray(
                np.asarray(inputs["up_inv_deg"][b], np.float32).T
            ).astype(BF_NP)
            for l in range(3):
                m[f"w{l}c"] = np.asarray(
                    inputs[f"l{l}_up_w"], np.float32).astype(BF_NP)
                m[f"w{l}b"] = np.asarray(
                    inputs[f"l{l}_bias"], np.float32)[:, :BH].astype(BF_NP)
        else:
            for l in range(3):
                m[f"prod{l}"] = np.ascontiguousarray(
                    (adj * wdn[l]).T).astype(BF_NP)
            m["invT"] = np.ascontiguousarray(
                np.asarray(inputs["down_inv_deg"][b], np.float32).T
            ).astype(BF_NP)
            for l in range(3):
                m[f"w{l}c"] = np.asarray(
                    inputs[f"l{l}_down_w"], np.float32).astype(BF_NP)
                m[f"w{l}b"] = np.asarray(
                    inputs[f"l{l}_bias"], np.float32)[:, BH:].astype(BF_NP)
        maps.append(m)
    return maps


_nc_cache = {}


def _get_program(n_cores, N):
    key = (n_cores, N)
    if key not in _nc_cache:
        _nc_cache[key] = build_program(n_cores, N)
    return _nc_cache[key]


def kernel(**inputs) -> np.ndarray:
    n_cores = 8
    nc = _get_program(n_cores, N_FULL)
    in_maps = make_in_maps(inputs, n_cores)
    res = run_bass_kernel_spmd(nc, in_maps, core_ids=list(range(n_cores)))
    out = np.zeros((B, 1), dtype=np.float32)
    for b in range(B):
        out[b, 0] = res.results[2 * b]["ypred"][0, 0]
    return out
